# revision 1
# baseline (speedup 1.0000x reference)
"""ARIMA mse_loss kernel for 8 Trainium2 NeuronCores (nn_ARIMA_59373627900097).

Math (validated against the jax reference):
  For each t in [33, S): window v = y[t-32:t], target y[t].
    mean = sum(v)/32 ; var = sum(v^2)/32 - mean^2 ; std = sqrt(var + 1e-5)
    err_t = dotG_t - C1*std_t
  where dotG_t = sum_j G_j y[t-32+j] is a 33-tap FIR folding the target, the
  AR filter (telescoped through d=1 differencing) and RevIN mean removal.

  On device, with s = 4*C1^2, dotG* = -sign(C1)*dotG and
  std' = sqrt(s*var + s*eps) = 2|C1|*std:
    -sign(C1)*err = dotG* + std'/2,   so   err^2 = (dotG* + std'/2)^2.
  The s/sqrt(s) scales are folded into the banded matmul weights on host;
  the bias tile carries s*eps.  The squared error lives in fp16 SBUF, so
  the final square-and-accumulate runs in the DVE 4x perf mode.

Sharding: time axis split over 8 cores, 131040 predicted timesteps each
(96 outputs x 1365 matmul columns); the 223-step remainder plus the head
term are computed on host in float64.

Device layout per core: X[p, c] = y_slice[96*c + p] (fp16, host-prepared).
Banded (128x96) filter matrices turn the mean/E2/dotG FIRs into TensorE
matmuls.  Per chunk (512/512/341 cols, first chunk split into two 256-col
matmul units for an earlier start):
  PE:      psA = Wmean^T X ; psC = G*^T X ; psB = WE2^T X^2   (fp16)
  GpSimd:  X^2 (fp16 TT); also issues one SWDGE input DMA
  ScalarE: m2 = Square(psA), std' = Sqrt(var' + bias)
  DVE:     var' = psB - m2 (STT), e = 0.5*std' + psC (STT, fp16 out),
           sq = e*e (all-SBUF fp16 STT @4x, accum -> acc column)
Inputs stream on three DMA queues (sync/scalar/gpsimd); the final (96,3)
accumulator DMA is issued without a completion wait -- the runtime drains
DMA queues before returning outputs.
"""

import numpy as np

P = 32
T0 = P + 1  # 33
S_TOTAL = 1048576
EPS_REVIN = 1e-5
EPS_W = 1e-10  # EPS*EPS in the reference denorm

N_CORES = 8
N_OUT = 96  # outputs per matmul column
N_COLS = 1365  # matmul columns per core
PER_CORE = N_OUT * N_COLS  # 131040 predicted timesteps per core
DATA_PER_CORE = 96 * (N_COLS - 1) + 128  # 131072 y values per core

_CACHED = {}


def _taps(ar_weight, ar_bias, rev_weight, rev_bias):
    """33-tap err filter G, plus C1 (std coefficient), in float64."""
    aw = np.asarray(ar_weight, np.float64).reshape(-1)
    ab = float(np.asarray(ar_bias).reshape(-1)[0])
    w = float(np.asarray(rev_weight).reshape(-1)[0])
    b = float(np.asarray(rev_bias).reshape(-1)[0])
    c = np.zeros(P)
    c[0] = aw[0] - aw[1]
    for j in range(1, P - 1):
        c[j] = aw[j] - aw[j + 1]
    c[P - 1] = aw[P - 1]
    c[P - 2] += -1.0  # dser[:, -1] term
    c[P - 1] += +1.0
    F = c - aw[0] / P
    A = ab + b * aw[0]
    C1 = (A - b) / (w + EPS_W)
    C2 = w / (w + EPS_W)
    G = np.zeros(P + 1)
    G[:P] = -(C2 * F + 1.0 / P)
    G[P] = 1.0
    return G, C1


def _band(taps):
    """(128, 96) banded filter matrix: W[o + j, o] = taps[j]."""
    ntap = len(taps)
    W = np.zeros((128, N_OUT), np.float32)
    for o in range(N_OUT):
        W[o : o + ntap, o] = taps
    return W


def _weights(ar_weight, ar_bias, rev_weight, rev_bias):
    """(W fp16 (128,288), bias f32 (128,1), G f64, C1 float)."""
    G, C1 = _taps(ar_weight, ar_bias, rev_weight, rev_bias)
    s = 4.0 * C1 * C1
    sq = np.sqrt(s)
    sgn = -1.0 if C1 > 0 else 1.0
    W = np.zeros((128, 288), np.float16)
    W[:, 0:96] = _band(np.full(P, sq / P)).astype(np.float16)  # mean' band
    W[:, 96:192] = _band(np.full(P, s / P)).astype(np.float16)  # E2' band
    W[:, 192:288] = _band((sgn * G).astype(np.float32)).astype(np.float16)
    bias = np.full((128, 1), s * EPS_REVIN, np.float32)
    return W, bias, G, C1


def _shard_x(yf):
    """Per-core fp16 X tiles: X[p, c] = y16[1 + k*PER_CORE + 96 c + p]."""
    y16 = yf.astype(np.float16)
    xs = []
    for k in range(N_CORES):
        start = 1 + k * PER_CORE
        data = y16[start : start + DATA_PER_CORE]
        v = np.lib.stride_tricks.as_strided(
            data, shape=(N_COLS, 128), strides=(96 * 2, 2)
        )
        xs.append(np.ascontiguousarray(v.T))  # (128, N_COLS)
    return xs


def _build_program():
    import concourse.bass as bass
    from concourse import mybir

    f16 = mybir.dt.float16
    f32 = mybir.dt.float32
    Alu = mybir.AluOpType
    Act = mybir.ActivationFunctionType

    nc = bass.Bass("TRN2", target_bir_lowering=False, debug=False,
                   num_devices=N_CORES)

    xd = nc.dram_tensor("x", [128, N_COLS], f16, kind="ExternalInput")
    wd = nc.dram_tensor("w", [128, 288], f16, kind="ExternalInput")
    cd = nc.dram_tensor("c1", [128, 1], f32, kind="ExternalInput")
    od = nc.dram_tensor("out", [96, 4], f32, kind="ExternalOutput")

    xs = nc.alloc_sbuf_tensor("xs", [128, N_COLS], f16)
    x2 = nc.alloc_sbuf_tensor("x2", [128, N_COLS], f16)
    ws = nc.alloc_sbuf_tensor("ws", [128, 288], f16)
    c1s = nc.alloc_sbuf_tensor("c1s", [128, 1], f32)
    acc = nc.alloc_sbuf_tensor("acc", [96, 4], f32)
    warm = nc.alloc_sbuf_tensor("warm", [128, 512], f16)  # uninitialized
    m2 = [nc.alloc_sbuf_tensor(f"m2_{s}", [96, 512], f16) for s in range(2)]
    var = [nc.alloc_sbuf_tensor(f"var_{s}", [96, 512], f16) for s in range(2)]
    std = [nc.alloc_sbuf_tensor(f"std_{s}", [96, 512], f16) for s in range(2)]
    # +/- err tiles: ea shared by chunks 0+3 (DVE-local), e1/e2 dedicated
    # (read cross-engine by ScalarE sq ops)
    ea = nc.alloc_sbuf_tensor("ea", [96, 512], f16)
    e1 = nc.alloc_sbuf_tensor("e1", [96, 512], f16)
    e2 = nc.alloc_sbuf_tensor("e2", [96, 512], f16)
    scr = nc.alloc_sbuf_tensor("scr", [96, 512], f16)      # sq dump

    psA = [nc.alloc_psum_tensor(f"psA{s}", [96, 512], f32) for s in range(2)]
    psB = [nc.alloc_psum_tensor(f"psB{s}", [96, 512], f32) for s in range(2)]
    psC = [nc.alloc_psum_tensor(f"psC{s}", [96, 512], f32) for s in range(2)]
    psD = nc.alloc_psum_tensor("psD", [96, 512], f32)  # warmup target

    c0f = nc.const_aps.tensor(0.0, (128, 1), f32)

    # chunks: (col0, ncols); psum set = chunk index % 2
    CH = [(0, 256), (256, 512), (768, 512), (1280, 85)]
    F = [256, 512, 512, 85]

    with (
        nc.Block() as block,
        nc.semaphore("s_dw") as s_dw,
        nc.semaphore("s_dc") as s_dc,
        nc.semaphore("s_dxa") as s_dxa,
        nc.semaphore("s_dxb") as s_dxb,
        nc.semaphore("s_dx2") as s_dx2,
        nc.semaphore("s_dx3") as s_dx3,
        nc.semaphore("s_pe") as s_pe,
        nc.semaphore("s_sc") as s_sc,
        nc.semaphore("s_v") as s_v,
        nc.semaphore("s_g") as s_g,
        nc.semaphore("s_do") as s_do,
    ):
        @block.sync
        def _(sync):
            sync.dma_start(
                out=xs.ap()[:, 0:256], in_=xd.ap()[:, 0:256]
            ).then_inc(s_dxa, 16)
            sync.dma_start(
                out=xs.ap()[:, 256:768], in_=xd.ap()[:, 256:768]
            ).then_inc(s_dxb, 16)
            sync.dma_start(out=c1s.ap(), in_=cd.ap()).then_inc(s_dc, 16)
            # final output DMA once every accumulator column is written;
            # no completion wait -- the runtime drains DMA queues before
            # the host reads outputs
            sync.wait_ge(s_v, 10)
            sync.wait_ge(s_sc, 10)
            sync.dma_start(out=od.ap(), in_=acc.ap()).then_inc(s_do, 16)

        @block.gpsimd
        def _(g):
            g.dma_start(
                out=xs.ap()[:, 1280:1365], in_=xd.ap()[:, 1280:1365]
            ).then_inc(s_dx3, 16)

            def x2_op(c0, fc, dxsem):
                g.wait_ge(dxsem, 16)
                g.tensor_tensor(
                    x2.ap()[:, c0 : c0 + fc], xs.ap()[:, c0 : c0 + fc],
                    xs.ap()[:, c0 : c0 + fc], Alu.mult,
                ).then_inc(s_g, 1)

            x2_op(0, 256, s_dxa)      # g1
            x2_op(256, 512, s_dxb)    # g2
            x2_op(768, 512, s_dx2)    # g3
            x2_op(1280, 85, s_dx3)    # g4

        @block.tensor
        def _(t):
            # warmup on uninitialized tiles keeps the PE activity window
            # hot through the DMA wait so real matmuls run above 1.2GHz;
            # the short final warmup quantizes any overrun into the real
            # matmul start
            for _ in range(4):
                t.matmul(psD.ap(), warm.ap()[:, 0:96], warm.ap(),
                         start=True, stop=True)
            t.matmul(psD.ap()[:, 0:160], warm.ap()[:, 0:96],
                     warm.ap()[:, 0:160], start=True, stop=True)
            t.wait_ge(s_dw, 16)

            def mm(dst, wcol, data, inc):
                t.matmul(dst, ws.ap()[:, wcol : wcol + 96], data,
                         start=True, stop=True).then_inc(s_pe, 1)

            # chunk 0: cols 0:256 (set 0)
            t.wait_ge(s_dxa, 16)
            mm(psA[0].ap()[:, 0:256], 0, xs.ap()[:, 0:256], 1)
            mm(psC[0].ap()[:, 0:256], 192, xs.ap()[:, 0:256], 2)
            t.wait_ge(s_g, 1)  # x2 chunk 0
            mm(psB[0].ap()[:, 0:256], 96, x2.ap()[:, 0:256], 3)
            # chunk 1: cols 256:768 (set 1)
            t.wait_ge(s_dxb, 16)
            mm(psA[1].ap()[:, 0:512], 0, xs.ap()[:, 256:768], 4)
            mm(psC[1].ap()[:, 0:512], 192, xs.ap()[:, 256:768], 5)
            t.wait_ge(s_g, 2)  # x2 chunk 1
            mm(psB[1].ap()[:, 0:512], 96, x2.ap()[:, 256:768], 6)
            # chunk 2: cols 768:1280 (set 0 reuse)
            t.wait_ge(s_dx2, 16)
            t.wait_ge(s_sc, 1)  # m2_0 read psA[0]
            mm(psA[0].ap()[:, 0:512], 0, xs.ap()[:, 768:1280], 7)
            t.wait_ge(s_v, 2)   # err_0 read psC[0]
            mm(psC[0].ap()[:, 0:512], 192, xs.ap()[:, 768:1280], 8)
            t.wait_ge(s_g, 3)   # x2 chunk 2
            mm(psB[0].ap()[:, 0:512], 96, x2.ap()[:, 768:1280], 9)
            # chunk 3: cols 1280:1365 (set 1 reuse)
            t.wait_ge(s_dx3, 16)
            t.wait_ge(s_sc, 2)  # m2_1 read psA[1]
            mm(psA[1].ap()[:, 0:85], 0, xs.ap()[:, 1280:1365], 10)
            t.wait_ge(s_v, 5)   # err_1 read psC[1]
            mm(psC[1].ap()[:, 0:85], 192, xs.ap()[:, 1280:1365], 11)
            t.wait_ge(s_g, 4)   # x2 chunk 3
            mm(psB[1].ap()[:, 0:85], 96, x2.ap()[:, 1280:1365], 12)

        @block.scalar
        def _(sc):
            sc.dma_start(out=ws.ap(), in_=wd.ap()).then_inc(s_dw, 16)
            sc.dma_start(
                out=xs.ap()[:, 768:1280], in_=xd.ap()[:, 768:1280]
            ).then_inc(s_dx2, 16)
            # dummy activation pulls the ACT table load off the critical path
            sc.activation(m2[0].ap()[:, 0:1], c0f[:96], Act.Square)

            def m2_op(ci, s, pe):
                fc = F[ci]
                sc.wait_ge(s_pe, pe)
                sc.activation(m2[s].ap()[:, :fc], psA[s].ap()[:, :fc],
                              Act.Square).then_inc(s_sc, 1)

            def std_op(ci, s, v):
                fc = F[ci]
                sc.wait_ge(s_v, v)
                sc.activation(std[s].ap()[:, :fc], var[s].ap()[:, :fc],
                              Act.Sqrt, bias=c1s.ap()[:96]).then_inc(s_sc, 1)

            def ssq_op(ci, et, vw):
                fc = F[ci]
                sc.wait_ge(s_v, vw)
                sc.activation(warm.ap()[:96, :fc], et.ap()[:, :fc],
                              Act.Square,
                              accum_out=acc.ap()[:, ci : ci + 1],
                              ).then_inc(s_sc, 1)

            m2_op(0, 0, 1)    # sc1 (after M1_0)
            m2_op(1, 1, 4)    # sc2 (after M1_1)
            sc.wait_ge(s_dc, 16)
            std_op(0, 0, 1)   # sc3 (after var_0 = v1)
            m2_op(2, 0, 7)    # sc4 (after M1_2)
            std_op(1, 1, 4)   # sc5 (after var_1 = v4)
            ssq_op(1, e1, 5)  # sc6 (after err_1 = v5)
            m2_op(3, 1, 10)   # sc7 (after M1_3)
            std_op(2, 0, 6)   # sc8 (after var_2 = v6)
            std_op(3, 1, 8)   # sc9 (after var_3 = v8)
            ssq_op(2, e2, 7)  # sc10 (after err_2 = v7)

        @block.vector
        def _(v):
            def var_op(ci, s, pe, scw):
                fc = F[ci]
                v.wait_ge(s_pe, pe)
                v.wait_ge(s_sc, scw)
                v.scalar_tensor_tensor(
                    var[s].ap()[:, :fc], m2[s].ap()[:, :fc], -1.0,
                    psB[s].ap()[:, :fc], Alu.mult, Alu.add,
                ).then_inc(s_v, 1)

            def err_op(ci, s, et, scw):
                fc = F[ci]
                v.wait_ge(s_sc, scw)
                v.scalar_tensor_tensor(
                    et.ap()[:, :fc], std[s].ap()[:, :fc], 0.5,
                    psC[s].ap()[:, :fc], Alu.mult, Alu.add,
                ).then_inc(s_v, 1)

            def sq_op(ci, et):
                fc = F[ci]
                v.scalar_tensor_tensor(
                    scr.ap()[:, :fc], et.ap()[:, :fc], 1.0,
                    et.ap()[:, :fc], Alu.bypass, Alu.mult,
                    accum_out=acc.ap()[:, ci : ci + 1],
                ).then_inc(s_v, 1)

            var_op(0, 0, 3, 1)        # v1: M3_0 + m2_0
            err_op(0, 0, ea, 3)       # v2: std_0
            sq_op(0, ea)              # v3
            var_op(1, 1, 6, 2)        # v4: M3_1 + m2_1
            err_op(1, 1, e1, 5)       # v5: std_1 (sq on ScalarE)
            var_op(2, 0, 9, 4)        # v6: M3_2 + m2_2
            err_op(2, 0, e2, 8)       # v7: std_2 = sc8 (sq on ScalarE)
            var_op(3, 1, 12, 7)       # v8: M3_3 + m2_3 = sc7
            err_op(3, 1, ea, 9)       # v9: std_3 = sc9
            sq_op(3, ea)              # v10

    return nc


def kernel(y, ar_weight, ar_bias, rev_weight, rev_bias):
    yf = np.asarray(y, np.float32).reshape(-1)
    S = yf.shape[0]
    assert S == S_TOTAL, f"kernel hardcoded for S={S_TOTAL}, got {S}"

    W, bias, G, C1 = _weights(ar_weight, ar_bias, rev_weight, rev_bias)
    xsh = _shard_x(yf)
    in_maps = [{"x": xsh[k], "w": W, "c1": bias} for k in range(N_CORES)]

    if "nc" not in _CACHED:
        _CACHED["nc"] = _build_program()
    nc = _CACHED["nc"]

    import os

    # keep run_bass_kernel_spmd on the plain (non-NTFF-trace) path; the
    # trace path needs hooks this container may not have installed
    os.environ["BASS_NEVER_TRACE"] = "1"
    from concourse.bass_utils import run_bass_kernel_spmd

    try:
        res = run_bass_kernel_spmd(nc, in_maps, list(range(N_CORES)))
    except Exception:
        # transient device/terminal hiccups happen; one retry
        import time

        time.sleep(5)
        res = run_bass_kernel_spmd(nc, in_maps, list(range(N_CORES)))

    total = 0.0
    for k in range(N_CORES):
        total += float(res.results[k]["out"].astype(np.float64).sum())

    y64 = yf.astype(np.float64)
    head = float((y64[:T0] ** 2).sum())

    # host tail: t in [33 + 8*PER_CORE, S)
    t_start = T0 + N_CORES * PER_CORE
    n_tail = S - t_start
    if n_tail > 0:
        idx = (t_start - P) + np.arange(n_tail)[:, None] + np.arange(P)[None, :]
        win = y64[idx]
        mean = win.mean(axis=1)
        varh = win.var(axis=1)
        stdv = np.sqrt(varh + EPS_REVIN)
        idx33 = (t_start - P) + np.arange(n_tail)[:, None] + np.arange(P + 1)[None, :]
        dotG = y64[idx33] @ G
        err = dotG - C1 * stdv
        total += float((err**2).sum())

    loss = (head + total) / S
    return np.array(loss, dtype=np.float32)

